# revision 14
# baseline (speedup 1.0000x reference)
"""Trainium2 Bass kernel for BrickVectorEdgeModel (GNN message passing).

Reference computation (per batch element b of 8):
  f  = relu(relu(x @ Wa + ba) @ Wb + bb)            # node MLP, x: [128, 256]
  e[i, j] = cat(f[j], f[i])                         # pairwise concat
  h1 = relu(e @ Wca + bca)                          # decomposed: G[j] + H[i]
  h2 = relu(h1 @ Wcb + bcb)
  h3 = relu(h2 @ Wcc + bcc)
  out[i, j] = h3 @ Wo + bo                          # [128, 128, 2]

Sharding: data-parallel over batch, one batch element per NeuronCore (8 cores).

Device kernel works in transposed activation layout [feat (partitions), cols]:
each layer is out_T[fo, col] = sum_k W[k, fo] * act_T[k, col], i.e.
matmul(psum, lhsT=W_chunk, rhs=actT_chunk), so activations never need an
on-chip transpose. The first edge layer is decomposed:
  h1_T[:, (i, j)] = relu(G_T[:, j] + (H_T[:, i] + bca))
which is a per-partition-scalar broadcast add + relu (one op per 128x128
block) instead of a [16384, 1024] x [1024, 512] matmul.

All matmuls run in bf16 with fp32 PSUM accumulation (measured end-to-end
scale-relative absmax error vs the fp32 reference: ~0.7%).

Trace-driven optimizations (vs the 288us baseline):
- Weights are host-packed into a few bundled dram tensors so the whole
  load is 8 DMA instructions instead of 19 (each dma_start costs ~600ns
  of HWDGE sequencer issue time on the sync engine, which serialized the
  transfer stream and pushed the last weight arrival past 17us).
- 8 warmup matmuls on a memset tile run during the weight-DMA window so
  the PE p-state (0.65 -> 1.2 -> 2.4 GHz ramp, ~3us) is at full clock
  when the real matmuls start.
- Explicit per-position PSUM bank tags (p0..p7, bufs=1 each). The
  default pool rotation handed cc(g) fo0 the bank freed by cb(g) fo3's
  ACT drain, whose chain (matmul end -> sem -> 687ns drain -> sem =
  887ns) is longer than the 864ns of out matmuls hiding it, stalling
  the PE ~100ns at every group boundary. With fixed tags every bank is
  reused a full group later (cc fo3 shares cb fo2's bank, 3us slack).
- h3 drains split fo0,fo3 -> DVE and fo1,fo2 -> ACT so the drain each
  out matmul needs lands with slack (ACT's queue reaches fo2 ~1.3us
  before out k=2 needs it; the old fo0,2->DVE split was just-in-time).
- G is computed before H (gated on the earlier Wcaj arrival) and the
  group-0 h1 build is split DVE/ACT so the first cb matmul isn't
  serialized behind 16 sequential DVE ops. (GpSimd measured 2.1us per
  tensor_scalar op here AND its SBUF traffic slowed concurrent DVE ops
  from ~250ns to 1-2us — keep it idle.)
"""

import numpy as np
import ml_dtypes

import concourse.bass as bass
import concourse.mybir as mybir
import concourse.tile as tile
from concourse import bacc
from concourse.bass_utils import run_bass_kernel_spmd

BF16 = mybir.dt.bfloat16
F32 = mybir.dt.float32

B = 8          # batch == number of cores
N = 128        # bricks per model (nodes)
D_IN = 256     # input feature dim
H = 512        # hidden dim
KA = D_IN // 128   # 2 input-feature chunks
C = H // 128       # 4 hidden-feature chunks
IG = 4             # i-values per group (4 * 128 cols = 512 = one PSUM bank)
NG = N // IG       # 32 groups

# Stashed by kernel() for harnesses that want profiling info (exec_time_ns
# is populated when BASS_TRACE=1 and the NTFF hook is available).
LAST_RESULTS = None


def _build_nc() -> bass.Bass:
    # Bacc (not raw Bass): its compile pass legalizes multi-wait sync_info
    # into forms walrus codegen accepts (raw Bass + Tile hits "Too many
    # sync wait commands" in CoreV2GenImpl setupSyncWait).
    nc = bacc.Bacc("TRN2", target_bir_lowering=False)

    # Bundled inputs (host packs, see kernel()):
    #   bundleA[:, 0:2, :]        xT k-chunks            (bf16)
    #   bundleA[:, 2+4k+fo, :]    Wa[k-chunk, fo-chunk]  (bf16)
    #   biasA[0, 0:512|512:1024|1024:1536]  ba | bb | bca (bf16)
    #   wbB[:, 4k+fo, :]          Wb                     (bf16)
    #   wcajB / wcaiB             Wca top / bottom half  (bf16)
    #   biasB[:, 0:4|4:8]         bcb | bcc              (f32)
    #   biasB[0:2, 8:9]           bo                     (f32)
    #   wdB[:, 4k+fo, :]          Wcb; [:, 16+4k+fo, :]  Wcc;
    #   wdB[:, 32, 2k:2k+2]       Wo k-chunk             (bf16)
    bundleA = nc.dram_tensor("bundleA", [128, 2 + 2 * C, 128], BF16, kind="ExternalInput")
    biasA = nc.dram_tensor("biasA", [1, 3 * H], BF16, kind="ExternalInput")
    wbB = nc.dram_tensor("wbB", [128, C * C, 128], BF16, kind="ExternalInput")
    wcajB = nc.dram_tensor("wcajB", [128, C * C, 128], BF16, kind="ExternalInput")
    biasB = nc.dram_tensor("biasB", [128, 2 * C + 1], F32, kind="ExternalInput")
    wcaiB = nc.dram_tensor("wcaiB", [128, C * C, 128], BF16, kind="ExternalInput")
    wdB = nc.dram_tensor("wdB", [128, 2 * C * C + C, 128], BF16, kind="ExternalInput")

    # Output in transposed layout [2, i, j]; host transposes to [i, j, 2].
    out = nc.dram_tensor("out", [2, N, N], F32, kind="ExternalOutput")

    relu = mybir.ActivationFunctionType.Relu
    ident = mybir.ActivationFunctionType.Identity
    add_op = mybir.AluOpType.add
    max_op = mybir.AluOpType.max

    with tile.TileContext(nc) as tc:
        with (
            tc.tile_pool(name="consts", bufs=1) as consts,
            tc.tile_pool(name="work", bufs=4) as work,
            tc.tile_pool(name="outp", bufs=6) as outp,
            tc.tile_pool(name="psmid", bufs=1, space="PSUM") as psmid,
        ):
            # Warmup source: memset immediately (no DMA dep) so the PE can
            # start ramping its p-state during the weight loads.
            warm_sb = consts.tile([128, 512], BF16, tag="warm_sb")
            nc.vector.memset(warm_sb, 0.0)
            ones_sb = consts.tile([1, N], BF16, tag="ones_sb")
            nc.vector.memset(ones_sb, 1.0)

            # ---- load constants: 8 DMAs on the sync HWDGE queue, in the
            # order compute needs them (transfers stream back-to-back).
            bundleA_sb = consts.tile([128, 2 + 2 * C, 128], BF16, tag="bundleA_sb")
            nc.sync.dma_start(out=bundleA_sb[:, 0:6], in_=bundleA[:, 0:6])
            biasA_sb = consts.tile([1, 3 * H], BF16, tag="biasA_sb")
            nc.sync.dma_start(out=biasA_sb, in_=biasA[:])
            nc.sync.dma_start(out=bundleA_sb[:, 6:], in_=bundleA[:, 6:])
            wb_sb = consts.tile([128, C * C, 128], BF16, tag="wb_sb")
            nc.sync.dma_start(out=wb_sb[:, 0:8], in_=wbB[:, 0:8])
            nc.sync.dma_start(out=wb_sb[:, 8:], in_=wbB[:, 8:])
            wcaj_sb = consts.tile([128, C * C, 128], BF16, tag="wcaj_sb")
            nc.sync.dma_start(out=wcaj_sb[:, 0:8], in_=wcajB[:, 0:8])
            nc.sync.dma_start(out=wcaj_sb[:, 8:], in_=wcajB[:, 8:])
            biasB_sb = consts.tile([128, 2 * C + 1], F32, tag="biasB_sb")
            nc.sync.dma_start(out=biasB_sb, in_=biasB[:])
            wcai_sb = consts.tile([128, C * C, 128], BF16, tag="wcai_sb")
            nc.sync.dma_start(out=wcai_sb[:, 0:8], in_=wcaiB[:, 0:8])
            nc.sync.dma_start(out=wcai_sb[:, 8:], in_=wcaiB[:, 8:])
            wd_sb = consts.tile([128, 2 * C * C + C, 128], BF16, tag="wd_sb")
            nc.sync.dma_start(out=wd_sb[:, 0:C * C], in_=wdB[:, 0:C * C])
            nc.sync.dma_start(out=wd_sb[:, C * C:], in_=wdB[:, C * C:])

            def wa(k, fo):
                return bundleA_sb[:, 2 + 2 * fo + k, :]

            def xin(k):
                return bundleA_sb[:, k, :]

            def wb(k, fo):
                return wb_sb[:, C * fo + k, :]

            def wcaj(k, fo):
                return wcaj_sb[:, C * fo + k, :]

            def wcai(k, fo):
                return wcai_sb[:, C * fo + k, :]

            def wcb(k, fo):
                return wd_sb[:, C * k + fo, :]

            def wcc(k, fo):
                return wd_sb[:, C * C + C * k + fo, :]

            def wo(k):
                # Wo k-chunk padded to a 128-wide stationary (cols 2:128 are
                # zero); keeps the PE's stationary geometry at 128 across the
                # out matmuls instead of dropping to M=2.
                return wd_sb[:, 2 * C * C + k, :]

            baT = biasA_sb[:, 0:H]
            bbT = biasA_sb[:, H:2 * H]
            bcaT = biasA_sb[:, 2 * H:3 * H]
            bcb_sb = biasB_sb[:, 0:C]
            bcc_sb = biasB_sb[:, C:2 * C]
            bo_sb = biasB_sb[0:2, 2 * C:2 * C + 1]

            # ---- PE warmup: 8 dummy matmuls (own psum bank, tag "pso")
            # ramp the tensor-engine clock during the DMA window.
            warm_ps = psmid.tile([128, 512], F32, tag="p4")
            for _ in range(6):
                nc.tensor.matmul(warm_ps, warm_sb[:, 0:128], warm_sb,
                                 start=True, stop=True)

            # ---- node MLP (tiny): f2_T = relu(Wb_T @ relu(Wa_T @ x_T + ba) + bb)
            # Each layer's 4 fo-chunks go to disjoint 128-col slices of ONE
            # psum bank so a single wide op drains the whole layer. The bias is
            # folded into the accumulation as a K=1 matmul against a ones row
            # (a start=True matmul clears has_written for the whole bank but
            # not the data, so sequential per-slice groups are safe).
            def node_layer(wfn, infn, kc, out_sb, biasT, func, ptag):
                pst = psmid.tile([128, C, N], F32, tag=ptag)
                for fo in range(C):
                    for k in range(kc):
                        nc.tensor.matmul(
                            pst[:, fo, :], wfn(k, fo), infn(k),
                            start=(k == 0), stop=False,
                        )
                    nc.tensor.matmul(
                        pst[:, fo, :], biasT[:, fo * 128:(fo + 1) * 128],
                        ones_sb, start=False, stop=True,
                    )
                nc.scalar.activation(out_sb[:, :, :], pst, func)

            f1_sb = consts.tile([128, C, N], BF16, tag="f1_sb")
            node_layer(wa, xin, KA, f1_sb, baT, relu, "p0")
            f2_sb = consts.tile([128, C, N], BF16, tag="f2_sb")
            node_layer(wb, lambda k: f1_sb[:, k, :], C, f2_sb, bbT, relu, "p1")

            # ---- G_T = Wcaj_T @ f2_T first (gated only on the earlier Wcaj
            # DMA; drains on DVE), then H'_T = Wcai_T @ f2_T + bca per chunk
            # with the group-0 h1 build fanned out over DVE/ACT/GpSimd.
            gt_sb = consts.tile([128, C, N], BF16, tag="gt_sb")
            ht_sb = consts.tile([128, C, N], F32, tag="ht_sb")
            h1_first = [work.tile([128, IG * N], BF16, tag=f"h1c{c}", name=f"h1f{c}")
                        for c in range(C)]
            for fo in range(C):
                pst = psmid.tile([128, N], F32, tag=f"p{fo}")
                for k in range(C):
                    nc.tensor.matmul(
                        pst, wcaj(k, fo), f2_sb[:, k, :],
                        start=(k == 0), stop=(k == C - 1),
                    )
                nc.scalar.copy(gt_sb[:, fo, :], pst)
            for fo in range(C):
                pst2 = psmid.tile([128, N], F32, tag=f"p{4 + fo}")
                for k in range(C):
                    nc.tensor.matmul(
                        pst2, wcai(k, fo), f2_sb[:, k, :],
                        start=(k == 0), stop=False,
                    )
                nc.tensor.matmul(
                    pst2, bcaT[:, fo * 128:(fo + 1) * 128], ones_sb,
                    start=False, stop=True,
                )
                nc.scalar.copy(ht_sb[:, fo, :], pst2)
                for il in range(IG):
                    dst = h1_first[fo][:, il * N:(il + 1) * N]
                    hsc = ht_sb[:, fo, il:il + 1]
                    if il == 0:
                        nc.scalar.activation(dst, gt_sb[:, fo, :], relu, bias=hsc)
                    else:
                        nc.vector.tensor_scalar(dst, gt_sb[:, fo, :], hsc,
                                                0.0, add_op, max_op)

            # ---- edge MLP over 32 groups of 4 i-values (512 cols each) ---------
            # h1-build for group g+1 is emitted mid-body (before group g's h3
            # drains) so the DVE FIFO runs it while the PE works on group g —
            # the next group's cb matmuls then start without waiting on DVE.
            def build_h1(g, act_assist=False):
                # h1[c][:, il*128 + j] = relu(G_T[c][:, j] + H'_T[c][:, g*IG+il])
                # For the first few groups (DVE backlog at the head) il=0 goes
                # to the ACT engine.
                t = [work.tile([128, IG * N], BF16, tag=f"h1c{c}", name=f"h1b{c}")
                     for c in range(C)]
                for c in range(C):
                    for il in range(IG):
                        dst = t[c][:, il * N:(il + 1) * N]
                        hsc = ht_sb[:, c, g * IG + il:g * IG + il + 1]
                        if act_assist and il == 0:
                            nc.scalar.activation(dst, gt_sb[:, c, :], relu,
                                                 bias=hsc)
                        else:
                            nc.vector.tensor_scalar(
                                dst, gt_sb[:, c, :], hsc, 0.0, add_op, max_op,
                            )
                return t

            # out_T[2, cols] = Wo_T @ h3 + bo for group g — emitted AFTER the
            # next group's cb matmuls so the PE never idles at a group
            # boundary waiting for h3 drains (the out matmuls used to block
            # ready cb work in the in-order PE queue). Its psum lives in the
            # dedicated "pso" bank so it never contends with cb/cc slots.
            def emit_out(g, h3_sb):
                pso = psmid.tile([128, IG * N], F32, tag="p4", name="pso")
                for k in range(C):
                    nc.tensor.matmul(
                        pso, wo(k), h3_sb[k],
                        start=(k == 0), stop=(k == C - 1),
                    )
                o_sb = outp.tile([2, IG, N], F32, tag="o_sb")
                nc.scalar.activation(o_sb, pso[0:2, :], ident, bias=bo_sb)
                eng = nc.scalar if g == NG - 1 else nc.sync
                eng.dma_start(out=out[:, g * IG:(g + 1) * IG, :], in_=o_sb)

            h1_next = h1_first
            h3_prev = None
            for g in range(NG):
                h1_sb = h1_next
                # Emit the next group's h1 build first: the DVE starts it
                # immediately (it has no deps on group g), keeping its work
                # out of the contended cc-phase window.
                if g + 1 < NG:
                    h1_next = build_h1(g + 1, act_assist=(g < 3))

                # h2 = relu(Wcb_T @ h1 + bcb)
                h2_sb = [work.tile([128, IG * N], BF16, tag=f"h2c{c}", name=f"h2c{c}")
                         for c in range(C)]
                for fo in range(C):
                    pst = psmid.tile([128, IG * N], F32, tag=f"p{fo}")
                    for k in range(C):
                        nc.tensor.matmul(
                            pst, wcb(k, fo), h1_sb[k],
                            start=(k == 0), stop=(k == C - 1),
                        )
                    nc.scalar.activation(h2_sb[fo], pst, relu,
                                         bias=bcb_sb[:, fo:fo + 1])

                if h3_prev is not None:
                    emit_out(g - 1, h3_prev)

                # h3 = relu(Wcc_T @ h2 + bcc); drains: fo0,fo3 -> DVE and
                # fo1,fo2 -> ACT, so the drains the next group's out matmuls
                # read land with slack on both engines.
                h3_sb = [work.tile([128, IG * N], BF16, tag=f"h3c{c}", name=f"h3c{c}")
                         for c in range(C)]
                cc_ptag = ["p5", "p6", "p7", "p2"]
                for fo in range(C):
                    pst = psmid.tile([128, IG * N], F32, tag=cc_ptag[fo])
                    for k in range(C):
                        nc.tensor.matmul(
                            pst, wcc(k, fo), h2_sb[k],
                            start=(k == 0), stop=(k == C - 1),
                        )
                    if fo in (0, 3):
                        nc.vector.tensor_scalar(
                            h3_sb[fo], pst, bcc_sb[:, fo:fo + 1], 0.0,
                            add_op, max_op,
                        )
                    else:
                        nc.scalar.activation(h3_sb[fo], pst, relu,
                                             bias=bcc_sb[:, fo:fo + 1])
                h3_prev = h3_sb

            emit_out(NG - 1, h3_prev)

    nc.compile()
    return nc


def _pack_w(w: np.ndarray) -> np.ndarray:
    """[K, F] f32 -> [128, K//128, F] bf16 so W[k, f] = packed[k % 128, k // 128, f]."""
    k, f = w.shape
    return np.ascontiguousarray(
        w.reshape(k // 128, 128, f).transpose(1, 0, 2)
    ).astype(ml_dtypes.bfloat16)


def _pack_w128(w: np.ndarray) -> np.ndarray:
    """[K, F] f32 -> [128, (K//128)*(F//128), 128] bf16, k-chunk-major then
    fo-chunk: slice [:, (F//128)*k + fo, :] is W[k-chunk, fo-chunk].T-packed."""
    k, f = w.shape
    p = _pack_w(w)                      # [128, K//128, F]
    return np.ascontiguousarray(
        p.reshape(128, k // 128, f // 128, 128).reshape(128, -1, 128)
    )


def _pack_w128_fo(w: np.ndarray) -> np.ndarray:
    """Like _pack_w128 but fo-chunk-major: slice [:, (K//128)*fo + k, :]."""
    k, f = w.shape
    p = _pack_w(w)                      # [128, K//128, F]
    return np.ascontiguousarray(
        p.reshape(128, k // 128, f // 128, 128).transpose(0, 2, 1, 3)
        .reshape(128, -1, 128)
    )


def _pack_b(b: np.ndarray) -> np.ndarray:
    """[F] f32 -> [128, F//128] f32 so b[f] = packed[f % 128, f // 128]."""
    return np.ascontiguousarray(b.reshape(-1, 128).T).astype(np.float32)


def kernel(brick_vectors, Wa, ba, Wb, bb, Wca, bca, Wcb, bcb, Wcc, bcc, Wo, bo):
    global LAST_RESULTS
    brick_vectors = np.asarray(brick_vectors, dtype=np.float32)

    bf16 = ml_dtypes.bfloat16
    wa_p = _pack_w128_fo(np.asarray(Wa))                  # [128, 8, 128]
    biasA = np.concatenate([
        np.asarray(ba), np.asarray(bb), np.asarray(bca)
    ]).reshape(1, 3 * H).astype(bf16)
    biasB = np.zeros((128, 2 * C + 1), dtype=np.float32)
    biasB[:, 0:C] = _pack_b(np.asarray(bcb))
    biasB[:, C:2 * C] = _pack_b(np.asarray(bcc))
    biasB[0:2, 2 * C] = np.asarray(bo, dtype=np.float32)
    wo_pad = np.zeros((128, C, 128), dtype=bf16)
    wo_pad[:, :, 0:2] = _pack_w(np.asarray(Wo))
    wdB = np.concatenate([
        _pack_w128(np.asarray(Wcb)), _pack_w128(np.asarray(Wcc)), wo_pad
    ], axis=1)

    shared = {
        "biasA": biasA,
        "wbB": _pack_w128_fo(np.asarray(Wb)),
        "wcajB": _pack_w128_fo(np.asarray(Wca)[:H]),
        "biasB": biasB,
        "wcaiB": _pack_w128_fo(np.asarray(Wca)[H:]),
        "wdB": wdB,
    }

    in_maps = []
    for b in range(B):
        xt = _pack_w(brick_vectors[b].T.astype(np.float32))   # [128, 2, 128]
        bundleA = np.concatenate([xt, wa_p], axis=1)          # [128, 10, 128]
        in_maps.append({"bundleA": bundleA, **shared})

    nc = _build_nc()
    res = run_bass_kernel_spmd(nc, in_maps, core_ids=list(range(B)))
    LAST_RESULTS = res

    out = np.empty((B, N, N, 2), dtype=np.float32)
    for b in range(B):
        out[b] = res.results[b]["out"].transpose(1, 2, 0)
    return out


# revision 15
# speedup vs baseline: 1.0017x; 1.0017x over previous
"""Trainium2 Bass kernel for BrickVectorEdgeModel (GNN message passing).

Reference computation (per batch element b of 8):
  f  = relu(relu(x @ Wa + ba) @ Wb + bb)            # node MLP, x: [128, 256]
  e[i, j] = cat(f[j], f[i])                         # pairwise concat
  h1 = relu(e @ Wca + bca)                          # decomposed: G[j] + H[i]
  h2 = relu(h1 @ Wcb + bcb)
  h3 = relu(h2 @ Wcc + bcc)
  out[i, j] = h3 @ Wo + bo                          # [128, 128, 2]

Sharding: data-parallel over batch, one batch element per NeuronCore (8 cores).

Device kernel works in transposed activation layout [feat (partitions), cols]:
each layer is out_T[fo, col] = sum_k W[k, fo] * act_T[k, col], i.e.
matmul(psum, lhsT=W_chunk, rhs=actT_chunk), so activations never need an
on-chip transpose. The first edge layer is decomposed:
  h1_T[:, (i, j)] = relu(G_T[:, j] + (H_T[:, i] + bca))
which is a per-partition-scalar broadcast add + relu (one op per 128x128
block) instead of a [16384, 1024] x [1024, 512] matmul.

All matmuls run in bf16 with fp32 PSUM accumulation (measured end-to-end
scale-relative absmax error vs the fp32 reference: ~0.7%).

Trace-driven optimizations (vs the 288us baseline):
- Weights are host-packed into a few bundled dram tensors so the whole
  load is 8 DMA instructions instead of 19 (each dma_start costs ~600ns
  of HWDGE sequencer issue time on the sync engine, which serialized the
  transfer stream and pushed the last weight arrival past 17us).
- 8 warmup matmuls on a memset tile run during the weight-DMA window so
  the PE p-state (0.65 -> 1.2 -> 2.4 GHz ramp, ~3us) is at full clock
  when the real matmuls start.
- Explicit per-position PSUM bank tags (p0..p7, bufs=1 each). The
  default pool rotation handed cc(g) fo0 the bank freed by cb(g) fo3's
  ACT drain, whose chain (matmul end -> sem -> 687ns drain -> sem =
  887ns) is longer than the 864ns of out matmuls hiding it, stalling
  the PE ~100ns at every group boundary. With fixed tags every bank is
  reused a full group later (cc fo3 shares cb fo2's bank, 3us slack).
- h3 drains split fo0,fo3 -> DVE and fo1,fo2 -> ACT so the drain each
  out matmul needs lands with slack (ACT's queue reaches fo2 ~1.3us
  before out k=2 needs it; the old fo0,2->DVE split was just-in-time).
- G is computed before H (gated on the earlier Wcaj arrival) and the
  group-0 h1 build is split DVE/ACT so the first cb matmul isn't
  serialized behind 16 sequential DVE ops. (GpSimd measured 2.1us per
  tensor_scalar op here AND its SBUF traffic slowed concurrent DVE ops
  from ~250ns to 1-2us — keep it idle.)
"""

import numpy as np
import ml_dtypes

import concourse.bass as bass
import concourse.mybir as mybir
import concourse.tile as tile
from concourse import bacc
from concourse.bass_utils import run_bass_kernel_spmd

BF16 = mybir.dt.bfloat16
F32 = mybir.dt.float32

B = 8          # batch == number of cores
N = 128        # bricks per model (nodes)
D_IN = 256     # input feature dim
H = 512        # hidden dim
KA = D_IN // 128   # 2 input-feature chunks
C = H // 128       # 4 hidden-feature chunks
IG = 4             # i-values per group (4 * 128 cols = 512 = one PSUM bank)
NG = N // IG       # 32 groups

# Stashed by kernel() for harnesses that want profiling info (exec_time_ns
# is populated when BASS_TRACE=1 and the NTFF hook is available).
LAST_RESULTS = None


def _build_nc() -> bass.Bass:
    # Bacc (not raw Bass): its compile pass legalizes multi-wait sync_info
    # into forms walrus codegen accepts (raw Bass + Tile hits "Too many
    # sync wait commands" in CoreV2GenImpl setupSyncWait).
    nc = bacc.Bacc("TRN2", target_bir_lowering=False)

    # Bundled inputs (host packs, see kernel()):
    #   bundleA[:, 0:2, :]        xT k-chunks            (bf16)
    #   bundleA[:, 2+4k+fo, :]    Wa[k-chunk, fo-chunk]  (bf16)
    #   biasA[0, 0:512|512:1024|1024:1536]  ba | bb | bca (bf16)
    #   wbB[:, 4k+fo, :]          Wb                     (bf16)
    #   wcajB / wcaiB             Wca top / bottom half  (bf16)
    #   biasB[:, 0:4|4:8]         bcb | bcc              (f32)
    #   biasB[0:2, 8:9]           bo                     (f32)
    #   wdB[:, 4k+fo, :]          Wcb; [:, 16+4k+fo, :]  Wcc;
    #   wdB[:, 32, 2k:2k+2]       Wo k-chunk             (bf16)
    bundleA = nc.dram_tensor("bundleA", [128, 2 + 2 * C, 128], BF16, kind="ExternalInput")
    biasA = nc.dram_tensor("biasA", [1, 3 * H], BF16, kind="ExternalInput")
    wbB = nc.dram_tensor("wbB", [128, C * C, 128], BF16, kind="ExternalInput")
    wcajB = nc.dram_tensor("wcajB", [128, C * C, 128], BF16, kind="ExternalInput")
    biasB = nc.dram_tensor("biasB", [128, 2 * C + 1], F32, kind="ExternalInput")
    wcaiB = nc.dram_tensor("wcaiB", [128, C * C, 128], BF16, kind="ExternalInput")
    wdB = nc.dram_tensor("wdB", [128, 2 * C * C + C, 128], BF16, kind="ExternalInput")

    # Output in transposed layout [2, i, j]; host transposes to [i, j, 2].
    out = nc.dram_tensor("out", [2, N, N], F32, kind="ExternalOutput")

    relu = mybir.ActivationFunctionType.Relu
    ident = mybir.ActivationFunctionType.Identity
    add_op = mybir.AluOpType.add
    max_op = mybir.AluOpType.max

    with tile.TileContext(nc) as tc:
        with (
            tc.tile_pool(name="consts", bufs=1) as consts,
            tc.tile_pool(name="work", bufs=4) as work,
            tc.tile_pool(name="outp", bufs=6) as outp,
            tc.tile_pool(name="psmid", bufs=1, space="PSUM") as psmid,
        ):
            # Warmup source: memset immediately (no DMA dep) so the PE can
            # start ramping its p-state during the weight loads.
            warm_sb = consts.tile([128, 512], BF16, tag="warm_sb")
            nc.vector.memset(warm_sb, 0.0)
            ones_sb = consts.tile([1, N], BF16, tag="ones_sb")
            nc.vector.memset(ones_sb, 1.0)

            # ---- load constants: 8 DMAs on the sync HWDGE queue, in the
            # order compute needs them (transfers stream back-to-back).
            bundleA_sb = consts.tile([128, 2 + 2 * C, 128], BF16, tag="bundleA_sb")
            nc.sync.dma_start(out=bundleA_sb, in_=bundleA[:])
            biasA_sb = consts.tile([1, 3 * H], BF16, tag="biasA_sb")
            nc.sync.dma_start(out=biasA_sb, in_=biasA[:])
            wb_sb = consts.tile([128, C * C, 128], BF16, tag="wb_sb")
            nc.sync.dma_start(out=wb_sb, in_=wbB[:])
            wcaj_sb = consts.tile([128, C * C, 128], BF16, tag="wcaj_sb")
            nc.sync.dma_start(out=wcaj_sb, in_=wcajB[:])
            biasB_sb = consts.tile([128, 2 * C + 1], F32, tag="biasB_sb")
            nc.sync.dma_start(out=biasB_sb, in_=biasB[:])
            wcai_sb = consts.tile([128, C * C, 128], BF16, tag="wcai_sb")
            nc.sync.dma_start(out=wcai_sb, in_=wcaiB[:])
            wd_sb = consts.tile([128, 2 * C * C + C, 128], BF16, tag="wd_sb")
            nc.sync.dma_start(out=wd_sb[:, 0:C * C], in_=wdB[:, 0:C * C])
            nc.sync.dma_start(out=wd_sb[:, C * C:], in_=wdB[:, C * C:])

            def wa(k, fo):
                return bundleA_sb[:, 2 + 2 * fo + k, :]

            def xin(k):
                return bundleA_sb[:, k, :]

            def wb(k, fo):
                return wb_sb[:, C * fo + k, :]

            def wcaj(k, fo):
                return wcaj_sb[:, C * fo + k, :]

            def wcai(k, fo):
                return wcai_sb[:, C * fo + k, :]

            def wcb(k, fo):
                return wd_sb[:, C * k + fo, :]

            def wcc(k, fo):
                return wd_sb[:, C * C + C * k + fo, :]

            def wo(k):
                # Wo k-chunk padded to a 128-wide stationary (cols 2:128 are
                # zero); keeps the PE's stationary geometry at 128 across the
                # out matmuls instead of dropping to M=2.
                return wd_sb[:, 2 * C * C + k, :]

            baT = biasA_sb[:, 0:H]
            bbT = biasA_sb[:, H:2 * H]
            bcaT = biasA_sb[:, 2 * H:3 * H]
            bcb_sb = biasB_sb[:, 0:C]
            bcc_sb = biasB_sb[:, C:2 * C]
            bo_sb = biasB_sb[0:2, 2 * C:2 * C + 1]

            # ---- PE warmup: 8 dummy matmuls (own psum bank, tag "pso")
            # ramp the tensor-engine clock during the DMA window.
            warm_ps = psmid.tile([128, 512], F32, tag="p4")
            for _ in range(6):
                nc.tensor.matmul(warm_ps, warm_sb[:, 0:128], warm_sb,
                                 start=True, stop=True)

            # ---- node MLP (tiny): f2_T = relu(Wb_T @ relu(Wa_T @ x_T + ba) + bb)
            # Each layer's 4 fo-chunks go to disjoint 128-col slices of ONE
            # psum bank so a single wide op drains the whole layer. The bias is
            # folded into the accumulation as a K=1 matmul against a ones row
            # (a start=True matmul clears has_written for the whole bank but
            # not the data, so sequential per-slice groups are safe).
            def node_layer(wfn, infn, kc, out_sb, biasT, func, ptag):
                pst = psmid.tile([128, C, N], F32, tag=ptag)
                for fo in range(C):
                    for k in range(kc):
                        nc.tensor.matmul(
                            pst[:, fo, :], wfn(k, fo), infn(k),
                            start=(k == 0), stop=False,
                        )
                    nc.tensor.matmul(
                        pst[:, fo, :], biasT[:, fo * 128:(fo + 1) * 128],
                        ones_sb, start=False, stop=True,
                    )
                nc.scalar.activation(out_sb[:, :, :], pst, func)

            f1_sb = consts.tile([128, C, N], BF16, tag="f1_sb")
            node_layer(wa, xin, KA, f1_sb, baT, relu, "p0")
            f2_sb = consts.tile([128, C, N], BF16, tag="f2_sb")
            node_layer(wb, lambda k: f1_sb[:, k, :], C, f2_sb, bbT, relu, "p1")

            # ---- G_T = Wcaj_T @ f2_T first (gated only on the earlier Wcaj
            # DMA; drains on DVE), then H'_T = Wcai_T @ f2_T + bca per chunk
            # with the group-0 h1 build fanned out over DVE/ACT/GpSimd.
            gt_sb = consts.tile([128, C, N], BF16, tag="gt_sb")
            ht_sb = consts.tile([128, C, N], F32, tag="ht_sb")
            h1_first = [work.tile([128, IG * N], BF16, tag=f"h1c{c}", name=f"h1f{c}")
                        for c in range(C)]
            for fo in range(C):
                pst = psmid.tile([128, N], F32, tag=f"p{fo}")
                for k in range(C):
                    nc.tensor.matmul(
                        pst, wcaj(k, fo), f2_sb[:, k, :],
                        start=(k == 0), stop=(k == C - 1),
                    )
                nc.scalar.copy(gt_sb[:, fo, :], pst)
            for fo in range(C):
                pst2 = psmid.tile([128, N], F32, tag=f"p{4 + fo}")
                for k in range(C):
                    nc.tensor.matmul(
                        pst2, wcai(k, fo), f2_sb[:, k, :],
                        start=(k == 0), stop=False,
                    )
                nc.tensor.matmul(
                    pst2, bcaT[:, fo * 128:(fo + 1) * 128], ones_sb,
                    start=False, stop=True,
                )
                nc.scalar.copy(ht_sb[:, fo, :], pst2)
                for il in range(IG):
                    dst = h1_first[fo][:, il * N:(il + 1) * N]
                    hsc = ht_sb[:, fo, il:il + 1]
                    if il == 0:
                        nc.scalar.activation(dst, gt_sb[:, fo, :], relu, bias=hsc)
                    else:
                        nc.vector.tensor_scalar(dst, gt_sb[:, fo, :], hsc,
                                                0.0, add_op, max_op)

            # ---- edge MLP over 32 groups of 4 i-values (512 cols each) ---------
            # h1-build for group g+1 is emitted mid-body (before group g's h3
            # drains) so the DVE FIFO runs it while the PE works on group g —
            # the next group's cb matmuls then start without waiting on DVE.
            def build_h1(g, act_assist=False):
                # h1[c][:, il*128 + j] = relu(G_T[c][:, j] + H'_T[c][:, g*IG+il])
                # For the first few groups (DVE backlog at the head) il=0 goes
                # to the ACT engine.
                t = [work.tile([128, IG * N], BF16, tag=f"h1c{c}", name=f"h1b{c}")
                     for c in range(C)]
                for c in range(C):
                    for il in range(IG):
                        dst = t[c][:, il * N:(il + 1) * N]
                        hsc = ht_sb[:, c, g * IG + il:g * IG + il + 1]
                        if act_assist and il == 0:
                            nc.scalar.activation(dst, gt_sb[:, c, :], relu,
                                                 bias=hsc)
                        else:
                            nc.vector.tensor_scalar(
                                dst, gt_sb[:, c, :], hsc, 0.0, add_op, max_op,
                            )
                return t

            # out_T[2, cols] = Wo_T @ h3 + bo for group g — emitted AFTER the
            # next group's cb matmuls so the PE never idles at a group
            # boundary waiting for h3 drains (the out matmuls used to block
            # ready cb work in the in-order PE queue). Its psum lives in the
            # dedicated "pso" bank so it never contends with cb/cc slots.
            def emit_out(g, h3_sb):
                pso = psmid.tile([128, IG * N], F32, tag="p4", name="pso")
                for k in range(C):
                    nc.tensor.matmul(
                        pso, wo(k), h3_sb[k],
                        start=(k == 0), stop=(k == C - 1),
                    )
                o_sb = outp.tile([2, IG, N], F32, tag="o_sb")
                nc.scalar.activation(o_sb, pso[0:2, :], ident, bias=bo_sb)
                nc.sync.dma_start(out=out[:, g * IG:(g + 1) * IG, :], in_=o_sb)

            h1_next = h1_first
            h3_prev = None
            for g in range(NG):
                h1_sb = h1_next
                # Emit the next group's h1 build first: the DVE starts it
                # immediately (it has no deps on group g), keeping its work
                # out of the contended cc-phase window.
                if g + 1 < NG:
                    h1_next = build_h1(g + 1, act_assist=(g < 3))

                # h2 = relu(Wcb_T @ h1 + bcb)
                h2_sb = [work.tile([128, IG * N], BF16, tag=f"h2c{c}", name=f"h2c{c}")
                         for c in range(C)]
                for fo in range(C):
                    pst = psmid.tile([128, IG * N], F32, tag=f"p{fo}")
                    for k in range(C):
                        nc.tensor.matmul(
                            pst, wcb(k, fo), h1_sb[k],
                            start=(k == 0), stop=(k == C - 1),
                        )
                    nc.scalar.activation(h2_sb[fo], pst, relu,
                                         bias=bcb_sb[:, fo:fo + 1])

                if h3_prev is not None:
                    emit_out(g - 1, h3_prev)

                # h3 = relu(Wcc_T @ h2 + bcc); drains: fo0,fo3 -> DVE and
                # fo1,fo2 -> ACT, so the drains the next group's out matmuls
                # read land with slack on both engines.
                h3_sb = [work.tile([128, IG * N], BF16, tag=f"h3c{c}", name=f"h3c{c}")
                         for c in range(C)]
                cc_ptag = ["p5", "p6", "p7", "p2"]
                for fo in range(C):
                    pst = psmid.tile([128, IG * N], F32, tag=cc_ptag[fo])
                    for k in range(C):
                        nc.tensor.matmul(
                            pst, wcc(k, fo), h2_sb[k],
                            start=(k == 0), stop=(k == C - 1),
                        )
                    if fo in (0, 3):
                        nc.vector.tensor_scalar(
                            h3_sb[fo], pst, bcc_sb[:, fo:fo + 1], 0.0,
                            add_op, max_op,
                        )
                    else:
                        nc.scalar.activation(h3_sb[fo], pst, relu,
                                             bias=bcc_sb[:, fo:fo + 1])
                h3_prev = h3_sb

            emit_out(NG - 1, h3_prev)

    nc.compile()
    return nc


def _pack_w(w: np.ndarray) -> np.ndarray:
    """[K, F] f32 -> [128, K//128, F] bf16 so W[k, f] = packed[k % 128, k // 128, f]."""
    k, f = w.shape
    return np.ascontiguousarray(
        w.reshape(k // 128, 128, f).transpose(1, 0, 2)
    ).astype(ml_dtypes.bfloat16)


def _pack_w128(w: np.ndarray) -> np.ndarray:
    """[K, F] f32 -> [128, (K//128)*(F//128), 128] bf16, k-chunk-major then
    fo-chunk: slice [:, (F//128)*k + fo, :] is W[k-chunk, fo-chunk].T-packed."""
    k, f = w.shape
    p = _pack_w(w)                      # [128, K//128, F]
    return np.ascontiguousarray(
        p.reshape(128, k // 128, f // 128, 128).reshape(128, -1, 128)
    )


def _pack_w128_fo(w: np.ndarray) -> np.ndarray:
    """Like _pack_w128 but fo-chunk-major: slice [:, (K//128)*fo + k, :]."""
    k, f = w.shape
    p = _pack_w(w)                      # [128, K//128, F]
    return np.ascontiguousarray(
        p.reshape(128, k // 128, f // 128, 128).transpose(0, 2, 1, 3)
        .reshape(128, -1, 128)
    )


def _pack_b(b: np.ndarray) -> np.ndarray:
    """[F] f32 -> [128, F//128] f32 so b[f] = packed[f % 128, f // 128]."""
    return np.ascontiguousarray(b.reshape(-1, 128).T).astype(np.float32)


def kernel(brick_vectors, Wa, ba, Wb, bb, Wca, bca, Wcb, bcb, Wcc, bcc, Wo, bo):
    global LAST_RESULTS
    brick_vectors = np.asarray(brick_vectors, dtype=np.float32)

    bf16 = ml_dtypes.bfloat16
    wa_p = _pack_w128_fo(np.asarray(Wa))                  # [128, 8, 128]
    biasA = np.concatenate([
        np.asarray(ba), np.asarray(bb), np.asarray(bca)
    ]).reshape(1, 3 * H).astype(bf16)
    biasB = np.zeros((128, 2 * C + 1), dtype=np.float32)
    biasB[:, 0:C] = _pack_b(np.asarray(bcb))
    biasB[:, C:2 * C] = _pack_b(np.asarray(bcc))
    biasB[0:2, 2 * C] = np.asarray(bo, dtype=np.float32)
    wo_pad = np.zeros((128, C, 128), dtype=bf16)
    wo_pad[:, :, 0:2] = _pack_w(np.asarray(Wo))
    wdB = np.concatenate([
        _pack_w128(np.asarray(Wcb)), _pack_w128(np.asarray(Wcc)), wo_pad
    ], axis=1)

    shared = {
        "biasA": biasA,
        "wbB": _pack_w128_fo(np.asarray(Wb)),
        "wcajB": _pack_w128_fo(np.asarray(Wca)[:H]),
        "biasB": biasB,
        "wcaiB": _pack_w128_fo(np.asarray(Wca)[H:]),
        "wdB": wdB,
    }

    in_maps = []
    for b in range(B):
        xt = _pack_w(brick_vectors[b].T.astype(np.float32))   # [128, 2, 128]
        bundleA = np.concatenate([xt, wa_p], axis=1)          # [128, 10, 128]
        in_maps.append({"bundleA": bundleA, **shared})

    nc = _build_nc()
    res = run_bass_kernel_spmd(nc, in_maps, core_ids=list(range(B)))
    LAST_RESULTS = res

    out = np.empty((B, N, N, 2), dtype=np.float32)
    for b in range(B):
        out[b] = res.results[b]["out"].transpose(1, 2, 0)
    return out
